# revision 1
# baseline (speedup 1.0000x reference)
"""BEV-pool (segment-sum scatter) Trainium2 kernel for nn_BaseDepthTransform.

Design:
  Host (numpy): replicate the reference geometry -> per-point flat BEV bin id
  (depends only on the small camera matrices, not on x). Sort points by bin.
  Greedily cut the sorted stream into "groups": up to KA*128 points spanning
  < W=16 distinct bins, each group = up to KA=8 point-tiles of 128. Binary-
  decompose group tile-counts into classes {8,4,2,1} so every class has a
  uniform static schedule. Ship, per core: a bf16 feature stream, a bf16
  per-tile one-hot stream ([128 points x 16 bins], built on host), laid out
  in DMA-friendly chunks.

  Device (Bass/Tile, SPMD x8): per group, chain c matmuls
  (one-hot^T @ feats) accumulating the group's [16,80] segment sums in PSUM,
  copy PSUM->SBUF on the Scalar engine, DMA to a per-group output slot.
  Only PE + ACT + DMA are used; no dynamic addressing, no collectives.

  Host reassembly: out[group] is added into grid[base:base+16] (groups may
  share bins across classes/cores; addition commutes).
"""
import sys
sys.path.insert(0, '/opt/trn_rl_repo')

import numpy as np
import ml_dtypes

BF16 = ml_dtypes.bfloat16

# ---- static problem config (mirrors the reference) ----
IH, IW = 256, 704
FH, FW = 32, 88
D = 118
C = 80
NXg, NYg, NZg = 360, 360, 1
BXc = np.array([-53.85, -53.85, 0.0], np.float32)
DXc = np.array([0.3, 0.3, 20.0], np.float32)
NBINS = NZg * NXg * NYg  # 129600
W = 16                   # bins per group window (arbitrary base)
KA = 8                   # max tiles per group / PSUM chain
NCORES = 8
CLASSES = (8, 4, 2, 1)
# groups per DMA chunk / PSUM wave, per class (24 slots = 4 banks, 12 = 2)
CHUNK_GROUPS = {8: 24, 4: 12, 2: 12, 1: 12}  # groups per DMA chunk
PSUM_SLOTS_PER_BANK = 6  # 6 x 80 f32 = 480 of 512
WAVE = 12                # groups per PSUM wave (2 banks)

_BUILD_CACHE = {}


def _frustum():
    ds = np.arange(1.0, 60.0, 0.5, dtype=np.float32)
    xs = np.linspace(0.0, IW - 1.0, FW, dtype=np.float32)
    ys = np.linspace(0.0, IH - 1.0, FH, dtype=np.float32)
    ds_g = np.broadcast_to(ds[:, None, None], (D, FH, FW))
    xs_g = np.broadcast_to(xs[None, None, :], (D, FH, FW))
    ys_g = np.broadcast_to(ys[None, :, None], (D, FH, FW))
    return np.stack([xs_g, ys_g, ds_g], axis=-1)  # [D,FH,FW,3]


def _get_geometry(c2l_rots, c2l_trans, intrins, post_rots, post_trans,
                  extra_rots, extra_trans):
    fr = _frustum()
    pts = fr[None, None] - post_trans[:, :, None, None, None, :]
    inv_pr = np.linalg.inv(post_rots).astype(np.float32)
    pts = np.einsum('bnij,bndhwj->bndhwi', inv_pr, pts).astype(np.float32)
    pts = np.concatenate([pts[..., :2] * pts[..., 2:3], pts[..., 2:3]], axis=-1)
    combine = np.einsum(
        'bnij,bnjk->bnik', c2l_rots, np.linalg.inv(intrins).astype(np.float32)
    ).astype(np.float32)
    pts = np.einsum('bnij,bndhwj->bndhwi', combine, pts).astype(np.float32)
    pts = pts + c2l_trans[:, :, None, None, None, :]
    pts = np.einsum('bij,bndhwj->bndhwi', extra_rots, pts).astype(np.float32)
    pts = pts + extra_trans[:, None, None, None, None, :]
    return pts  # [B,N,D,FH,FW,3]


def _flat_bins(geom):
    """Per-point flat bin id (int64), -1 for dropped points."""
    coords = ((geom - (BXc - DXc / 2.0)) / DXc).astype(np.int32)
    B = coords.shape[0]
    coords = coords.reshape(B, -1, 3)
    cx, cy, cz = coords[..., 0], coords[..., 1], coords[..., 2]
    kept = (cx >= 0) & (cx < NXg) & (cy >= 0) & (cy < NYg) & (cz >= 0) & (cz < NZg)
    flat = ((cz.astype(np.int64) * NXg + cx) * NYg + cy)
    flat = np.where(kept, flat, -1)
    return flat  # [B, Np]


def _round_up(x, m):
    return ((x + m - 1) // m) * m


def _cut_groups(fk_sorted):
    """Greedy: groups of <=KA*128 points spanning < W bins, binary-decomposed
    into class segments [(cls, start, npts, base), ...] in stream order."""
    n = len(fk_sorted)
    segs = []
    i = 0
    while i < n:
        hi = np.searchsorted(fk_sorted, fk_sorted[i] + W, side='left')
        j = min(i + KA * 128, hi, n)
        npts = j - i
        base = int(fk_sorted[i])
        nt = (npts + 127) // 128
        s = i
        for c in CLASSES:
            while nt >= c:
                ln = min(c * 128, j - s)
                segs.append((c, s, ln, base))
                s += ln
                nt -= c
        i = j
    return segs


def _split_classes(segs):
    """Per class: contiguous split across cores balanced by group count,
    padded to uniform per-class counts. {cls: (percore seg lists, Gmax)}."""
    out = {}
    for c in CLASSES:
        cl = [s for s in segs if s[0] == c]
        G = len(cl)
        per = []
        for ci in range(NCORES):
            lo = (G * ci) // NCORES
            hi = (G * (ci + 1)) // NCORES
            per.append(cl[lo:hi])
        Gmax = max(1, max(len(p) for p in per))
        Gmax = _round_up(Gmax, CHUNK_GROUPS[c])
        out[c] = (per, Gmax)
    return out


def _build_core_inputs(class_split, fk_sorted, pidx_sorted, xflat_bf):
    """Build per-core input dict: per class feats + onehot streams."""
    maps = [dict() for _ in range(NCORES)]
    meta = {c: [] for c in CLASSES}  # per class: percore array of bases
    for c in CLASSES:
        per, Gmax = class_split[c]
        T = Gmax * c
        for ci in range(NCORES):
            segs = per[ci]
            feats = np.zeros((T, 128, C), BF16)
            oh = np.zeros((T, 128, W), BF16)
            bases = np.full((Gmax,), -1, np.int64)
            for gi, (_, s, ln, base) in enumerate(segs):
                bases[gi] = base
                lids = (fk_sorted[s:s + ln] - base).astype(np.int64)
                pix = pidx_sorted[s:s + ln]
                t0 = gi * c
                nt = (ln + 127) // 128
                for k in range(nt):
                    a, b = k * 128, min((k + 1) * 128, ln)
                    m = b - a
                    feats[t0 + k, :m] = xflat_bf[pix[a:b]]
                    oh[t0 + k, np.arange(m), lids[a:b]] = 1
            CH = CHUNK_GROUPS[c]
            nch = Gmax // CH
            f = feats.reshape(nch, CH * c, 128, C).transpose(0, 2, 1, 3) \
                     .reshape(nch, 128, CH * c * C)
            o8 = oh.astype(ml_dtypes.float8_e4m3)
            o = o8.reshape(nch, CH * c, 128, W).transpose(0, 2, 1, 3) \
                  .reshape(nch, 128, CH * c * W)
            maps[ci][f"feats{c}"] = np.ascontiguousarray(f)
            maps[ci][f"oh{c}"] = np.ascontiguousarray(o)
            meta[c].append(bases)
    return maps, meta


def _build_bass(shape_key):
    """shape_key: tuple of (cls, Gmax) pairs."""
    if shape_key in _BUILD_CACHE:
        return _BUILD_CACHE[shape_key]
    from concourse import bass, mybir, tile, bacc

    nc = bacc.Bacc()
    params = {}
    for c, Gmax in shape_key:
        CH = CHUNK_GROUPS[c]
        nch = Gmax // CH
        params[f"feats{c}"] = nc.declare_dram_parameter(
            f"feats{c}", [nch, 128, CH * c * C], mybir.dt.bfloat16,
            isOutput=False)
        params[f"oh{c}"] = nc.declare_dram_parameter(
            f"oh{c}", [nch, 128, CH * c * W], mybir.dt.float8e4,
            isOutput=False)
        params[f"out{c}"] = nc.declare_dram_parameter(
            f"out{c}", [W, Gmax, C], mybir.dt.float32, isOutput=True)

    def slot_off(s):
        return (s // PSUM_SLOTS_PER_BANK) * 512 + (s % PSUM_SLOTS_PER_BANK) * C

    # interleave class chunks so short small-class pipelines hide under the
    # dense class-8 stream
    chunk_order = []
    for c, Gmax in shape_key:
        nch = Gmax // CHUNK_GROUPS[c]
        for ch in range(nch):
            chunk_order.append((c, Gmax, ch, (ch + 0.5) / nch))
    chunk_order.sort(key=lambda t: t[3])

    with tile.TileContext(nc) as tc:
        with tc.tile_pool(name="fstream", bufs=5) as fpool, \
             tc.tile_pool(name="stage", bufs=8) as spool, \
             tc.tile_pool(name="psum", bufs=4, space="PSUM") as psum_pool:
            for c, Gmax, ch, _frac in chunk_order:
                CH = CHUNK_GROUPS[c]
                nch = Gmax // CH
                fchunk = fpool.tile([128, CH * c * C], mybir.dt.bfloat16,
                                    tag="fchunk")
                nc.sync.dma_start(fchunk[:], params[f"feats{c}"][ch, :, :])
                ochunk = fpool.tile([128, CH * c * W], mybir.dt.float8e4,
                                    tag="ochunk")
                nc.scalar.dma_start(ochunk[:], params[f"oh{c}"][ch, :, :])
                nwave = (CH + WAVE - 1) // WAVE
                for wv in range(nwave):
                    g0 = wv * WAVE
                    NW = min(WAVE, CH - g0)
                    nbank = NW // PSUM_SLOTS_PER_BANK
                    mega = psum_pool.tile([W, nbank * 512], mybir.dt.float32,
                                          tag="ps")
                    half = NW // 2
                    for gp in range(half):
                        ga, gb = g0 + gp, g0 + gp + half
                        oa, ob = slot_off(gp), slot_off(gp + half)
                        for k in range(c):
                            ta = ga * c + k
                            tb = gb * c + k
                            nc.tensor.matmul(
                                out=mega[:, oa:oa + C],
                                lhsT=ochunk[:, ta * W:(ta + 1) * W],
                                rhs=fchunk[:, ta * C:(ta + 1) * C],
                                start=(k == 0), stop=(k == c - 1))
                            nc.tensor.matmul(
                                out=mega[:, ob:ob + C],
                                lhsT=ochunk[:, tb * W:(tb + 1) * W],
                                rhs=fchunk[:, tb * C:(tb + 1) * C],
                                start=(k == 0), stop=(k == c - 1))
                    st = spool.tile([W, NW, C], mybir.dt.float32, tag="st")
                    src_ap = bass.AP(
                        mega[:].tensor, mega[:].offset,
                        [mega[:].ap[0], [512, nbank],
                         [C, PSUM_SLOTS_PER_BANK], [1, C]])
                    dst_ap = bass.AP(
                        st[:].tensor, st[:].offset,
                        [st[:].ap[0], [PSUM_SLOTS_PER_BANK * C, nbank],
                         [C, PSUM_SLOTS_PER_BANK], [1, C]])
                    nc.scalar.copy(dst_ap, src_ap)
                    nc.scalar.dma_start(
                        params[f"out{c}"][:, ch * CH + g0:ch * CH + g0 + NW, :],
                        st[:])
    nc.finalize()
    _BUILD_CACHE[shape_key] = nc
    return nc


def run_scheduled(x, flat, trace=False, trace_cores=None):
    """Core pipeline given precomputed flat bins; returns (grid, results)."""
    from concourse.bass_utils import run_bass_kernel_spmd

    xflat_bf = np.ascontiguousarray(x.reshape(-1, C)).astype(BF16)
    kept_idx = np.nonzero(flat >= 0)[0]
    fk = flat[kept_idx]
    order = np.argsort(fk, kind='stable')
    fk_sorted = fk[order]
    pidx_sorted = kept_idx[order]

    segs = _cut_groups(fk_sorted)
    class_split = _split_classes(segs)
    shape_key = tuple((c, class_split[c][1]) for c in CLASSES)

    maps, meta = _build_core_inputs(class_split, fk_sorted, pidx_sorted,
                                    xflat_bf)
    nc = _build_bass(shape_key)
    res = run_bass_kernel_spmd(nc, maps, core_ids=list(range(NCORES)),
                               trace=trace, trace_cores=trace_cores)

    grid = np.zeros((NBINS + W, C), np.float32)
    for c in CLASSES:
        for ci in range(NCORES):
            outs = res.results[ci][f"out{c}"]   # [W, Gmax, C]
            bases = meta[c][ci]
            for gi in range(len(bases)):
                base = bases[gi]
                if base >= 0:
                    grid[base:base + W] += outs[:, gi]
    return grid[:NBINS], res


def kernel(x, camera2lidar_rots, camera2lidar_trans, intrins, post_rots,
           post_trans, extra_rots, extra_trans):
    x = np.asarray(x, np.float32)
    B, N = x.shape[0], x.shape[1]
    assert (B, N) == (1, 6) and x.shape[2:] == (D, FH, FW, C), x.shape

    geom = _get_geometry(
        np.asarray(camera2lidar_rots, np.float32),
        np.asarray(camera2lidar_trans, np.float32),
        np.asarray(intrins, np.float32),
        np.asarray(post_rots, np.float32),
        np.asarray(post_trans, np.float32),
        np.asarray(extra_rots, np.float32),
        np.asarray(extra_trans, np.float32),
    )
    flat = _flat_bins(geom)[0]          # [Np]
    grid, _ = run_scheduled(x, flat)
    outp = grid.reshape(NXg, NYg, C).transpose(2, 0, 1)[None]  # [1,C,NX,NY]
    return np.ascontiguousarray(outp)



# revision 11
# speedup vs baseline: 1.7449x; 1.7449x over previous
"""BEV-pool (segment-sum scatter) Trainium2 kernel for nn_BaseDepthTransform.

Design:
  Host (numpy): replicate the reference geometry -> per-point flat BEV bin id
  (depends only on the small camera matrices, not on x). Sort points by bin.
  Greedily cut the sorted stream into "groups": up to KA*128 points spanning
  < W=16 distinct bins, each group = up to KA=8 point-tiles of 128. Binary-
  decompose group tile-counts into classes {8,4,2,1} so every class has a
  uniform static schedule across the 8 SPMD cores. Ship, per core and class:
  a bf16 feature stream [128, T*C] and a u8 local-bin-id stream [128, T]
  (one byte per point; 255 for pad rows).

  Features ship as fp8e4 with per-(bin,channel) ERROR FEEDBACK applied on
  the host along the sorted stream (q_i = fp8(x_i + carry)); the device
  sums the quantized values exactly (fp8 one-hot x fp8 feats -> f32 PSUM),
  so each bin's total error collapses to the last point's residual instead
  of growing sqrt(m). Measured end-to-end rel err ~3e-3 (gate 2e-2).

  Device (Bass/Tile, SPMD x8): build the [128, W] one-hots on-chip with a
  single DVE is_equal against an iota constant (stride-0 broadcast APs),
  then per group chain c/2 DoubleRow matmuls (each contracts TWO 128-pt
  tiles, 0.5 cycles/row) accumulating the group's [16,80] segment sums in
  PSUM. Waves of up to 12 groups fill 2 PSUM banks (even slots bank0, odd
  slots bank1 so paired chains interleave); evictions PSUM->SBUF (cast to
  bf16) alternate between the Scalar and Vector engines, then DMA out.
  No dynamic addressing, no collectives.

  Host reassembly: out[group] is added into grid[base:base+16] (groups may
  share bins across classes/cores; addition commutes).
"""
import sys
sys.path.insert(0, '/opt/trn_rl_repo')

import numpy as np
import ml_dtypes

BF16 = ml_dtypes.bfloat16

# ---- static problem config (mirrors the reference) ----
IH, IW = 256, 704
FH, FW = 32, 88
D = 118
C = 80
NXg, NYg, NZg = 360, 360, 1
BXc = np.array([-53.85, -53.85, 0.0], np.float32)
DXc = np.array([0.3, 0.3, 20.0], np.float32)
NBINS = NZg * NXg * NYg  # 129600
W = 16                   # bins per group window
KA = 8                   # max tiles per group / PSUM chain
NCORES = 8
CLASSES = (8, 4, 2, 1)
WAVE = 12                # groups per PSUM wave (2 banks, 6 slots each)

_BUILD_CACHE = {}


def _frustum():
    ds = np.arange(1.0, 60.0, 0.5, dtype=np.float32)
    xs = np.linspace(0.0, IW - 1.0, FW, dtype=np.float32)
    ys = np.linspace(0.0, IH - 1.0, FH, dtype=np.float32)
    ds_g = np.broadcast_to(ds[:, None, None], (D, FH, FW))
    xs_g = np.broadcast_to(xs[None, None, :], (D, FH, FW))
    ys_g = np.broadcast_to(ys[None, :, None], (D, FH, FW))
    return np.stack([xs_g, ys_g, ds_g], axis=-1)  # [D,FH,FW,3]


def _get_geometry(c2l_rots, c2l_trans, intrins, post_rots, post_trans,
                  extra_rots, extra_trans):
    fr = _frustum()
    pts = fr[None, None] - post_trans[:, :, None, None, None, :]
    inv_pr = np.linalg.inv(post_rots).astype(np.float32)
    pts = np.einsum('bnij,bndhwj->bndhwi', inv_pr, pts).astype(np.float32)
    pts = np.concatenate([pts[..., :2] * pts[..., 2:3], pts[..., 2:3]], axis=-1)
    combine = np.einsum(
        'bnij,bnjk->bnik', c2l_rots, np.linalg.inv(intrins).astype(np.float32)
    ).astype(np.float32)
    pts = np.einsum('bnij,bndhwj->bndhwi', combine, pts).astype(np.float32)
    pts = pts + c2l_trans[:, :, None, None, None, :]
    pts = np.einsum('bij,bndhwj->bndhwi', extra_rots, pts).astype(np.float32)
    pts = pts + extra_trans[:, None, None, None, None, :]
    return pts  # [B,N,D,FH,FW,3]


def _flat_bins(geom):
    """Per-point flat bin id (int64), -1 for dropped points."""
    coords = ((geom - (BXc - DXc / 2.0)) / DXc).astype(np.int32)
    B = coords.shape[0]
    coords = coords.reshape(B, -1, 3)
    cx, cy, cz = coords[..., 0], coords[..., 1], coords[..., 2]
    kept = (cx >= 0) & (cx < NXg) & (cy >= 0) & (cy < NYg) & (cz >= 0) & (cz < NZg)
    flat = ((cz.astype(np.int64) * NXg + cx) * NYg + cy)
    flat = np.where(kept, flat, -1)
    return flat  # [B, Np]


def _round_up(x, m):
    return ((x + m - 1) // m) * m


def _cut_groups(fk_sorted):
    """Greedy: groups of <=KA*128 points spanning < W bins, binary-decomposed
    into class segments [(cls, start, npts, base), ...] in stream order."""
    n = len(fk_sorted)
    segs = []
    i = 0
    while i < n:
        hi = np.searchsorted(fk_sorted, fk_sorted[i] + W, side='left')
        j = min(i + KA * 128, hi, n)
        npts = j - i
        base = int(fk_sorted[i])
        nt = (npts + 127) // 128
        s = i
        for c in CLASSES:
            while nt >= c:
                ln = min(c * 128, j - s)
                segs.append((c, s, ln, base))
                s += ln
                nt -= c
        i = j
    return segs


def _split_classes(segs):
    """Per class: contiguous split across cores balanced by group count,
    padded to uniform per-class counts (rounded to 2 only).
    {cls: (percore seg lists, Gmax)}."""
    out = {}
    for c in CLASSES:
        cl = [s for s in segs if s[0] == c]
        G = len(cl)
        per = []
        for ci in range(NCORES):
            lo = (G * ci) // NCORES
            hi = (G * (ci + 1)) // NCORES
            per.append(cl[lo:hi])
        Gmax = max(2, _round_up(max(len(p) for p in per), 2))
        out[c] = (per, Gmax)
    return out


def _class_chunks(Gmax):
    """List of (gstart, NW) waves; NW == WAVE except an even tail."""
    chunks = []
    g = 0
    while g + WAVE <= Gmax:
        chunks.append((g, WAVE))
        g += WAVE
    if g < Gmax:
        chunks.append((g, Gmax - g))
    return chunks


def _fb_quant(vals, starts, counts):
    """fp8e4 quantization of the bin-sorted stream with per-(bin,channel)
    error feedback: q_i = fp8(x_i + carry); carry = x_i + carry - q_i.
    The shipped stream then satisfies sum_bin(q) = sum_bin(x) - last_carry."""
    FP8 = ml_dtypes.float8_e4m3
    order = np.argsort(-counts, kind='stable')  # bins by count desc
    starts_s = np.ascontiguousarray(starts[order])
    counts_s = np.ascontiguousarray(counts[order])
    maxm = int(counts_s[0]) if len(counts_s) else 0
    q = np.empty(vals.shape, FP8)
    carry = np.zeros((len(starts_s), C), np.float32)
    neg = -counts_s.astype(np.int64)
    for k in range(maxm):
        n_k = np.searchsorted(neg, -(k), side='left')  # bins with count > k
        if n_k == 0:
            break
        idx = starts_s[:n_k] + k
        v = vals[idx] + carry[:n_k]
        qk = v.astype(FP8)
        q[idx] = qk
        carry[:n_k] = v - qk.astype(np.float32)
    return q


def _build_core_inputs(class_split, fk_sorted, qvals_sorted):
    """Build per-core input dict: per class feats [128,T*C] fp8e4 and
    lids [128,T] u8 (255 = pad row)."""
    FP8 = ml_dtypes.float8_e4m3
    maps = [dict() for _ in range(NCORES)]
    meta = {c: [] for c in CLASSES}  # per class: percore array of bases
    iota = np.broadcast_to(np.arange(W, dtype=np.uint8), (128, W))
    for c in CLASSES:
        per, Gmax = class_split[c]
        T = Gmax * c
        for ci in range(NCORES):
            segs = per[ci]
            feats = np.zeros((T, 128, C), FP8)
            lids = np.full((T, 128), 255, np.uint8)
            bases = np.full((Gmax,), -1, np.int64)
            for gi, (_, s, ln, base) in enumerate(segs):
                bases[gi] = base
                lid = (fk_sorted[s:s + ln] - base).astype(np.uint8)
                t0 = gi * c
                nt = (ln + 127) // 128
                for k in range(nt):
                    a, b = k * 128, min((k + 1) * 128, ln)
                    m = b - a
                    feats[t0 + k, :m] = qvals_sorted[s + a:s + b]
                    lids[t0 + k, :m] = lid[a:b]
            maps[ci][f"feats{c}"] = np.ascontiguousarray(
                feats.transpose(1, 0, 2).reshape(128, T * C))
            maps[ci][f"lid{c}"] = np.ascontiguousarray(lids.T)
            meta[c].append(bases)
    for ci in range(NCORES):
        maps[ci]["iota16"] = np.ascontiguousarray(iota)
    return maps, meta


def _build_bass(shape_key):
    """shape_key: tuple of (cls, Gmax) pairs."""
    if shape_key in _BUILD_CACHE:
        return _BUILD_CACHE[shape_key]
    from concourse import bass, mybir, tile, bacc

    nc = bacc.Bacc()
    params = {}
    for c, Gmax in shape_key:
        T = Gmax * c
        params[f"feats{c}"] = nc.declare_dram_parameter(
            f"feats{c}", [128, T * C], mybir.dt.float8e4, isOutput=False)
        params[f"lid{c}"] = nc.declare_dram_parameter(
            f"lid{c}", [128, T], mybir.dt.uint8, isOutput=False)
        params[f"out{c}"] = nc.declare_dram_parameter(
            f"out{c}", [W, Gmax, C], mybir.dt.bfloat16, isOutput=True)
    params["iota16"] = nc.declare_dram_parameter(
        "iota16", [128, W], mybir.dt.uint8, isOutput=False)

    # interleave class waves so short small-class pipelines hide under the
    # dense class-8 stream
    chunk_order = []
    for c, Gmax in shape_key:
        chunks = _class_chunks(Gmax)
        n = len(chunks)
        for idx, (gs, nw) in enumerate(chunks):
            chunk_order.append((c, gs, nw, (idx + 0.5) / n))
    chunk_order.sort(key=lambda t: t[3])

    with tile.TileContext(nc) as tc:
        with tc.tile_pool(name="fstream", bufs=10) as fpool, \
             tc.tile_pool(name="psum", bufs=4, space="PSUM") as psum_pool:
            # issue the first feats chunk before anything else so the DMA
            # engines start pulling immediately; lids/iota follow on sync
            # while scalar covers the next chunks
            c0, gs0, NW0, _ = None, None, None, None
            fch_head = None
            if chunk_order:
                c0, gs0, NW0, _frac0 = chunk_order[0]
                fch_head = fpool.tile([128, NW0 * c0 * C], mybir.dt.float8e4,
                                      tag="fchunk", name="fch_head")
                nc.sync.dma_start(
                    fch_head[:],
                    params[f"feats{c0}"][:, gs0 * c0 * C:(gs0 + NW0) * c0 * C])
            iota_t = fpool.tile([128, W], mybir.dt.uint8, tag="iota", bufs=1)
            nc.sync.dma_start(iota_t[:], params["iota16"][:, :])
            lid_tiles = {}
            for c, Gmax in shape_key:
                lt = fpool.tile([128, Gmax * c], mybir.dt.uint8,
                                tag=f"lid{c}", bufs=1, name=f"lidt{c}")
                nc.sync.dma_start(lt[:], params[f"lid{c}"][:, :])
                lid_tiles[c] = lt

            def dr_aps(och, fch, t0):
                """lhsT/rhs APs for a DoubleRow matmul over tiles t0, t0+1."""
                lt = och[:, t0 * W:(t0 + 2) * W]
                rt = fch[:, t0 * C:(t0 + 2) * C]
                lhsT = bass.AP(lt.tensor, lt.offset,
                               [lt.ap[0], [W, 2], [1, W]])
                rhs = bass.AP(rt.tensor, rt.offset,
                              [rt.ap[0], [C, 2], [1, C]])
                return lhsT, rhs

            for wv_idx, (c, gs, NW, _frac) in enumerate(chunk_order):
                NT = NW * c
                if wv_idx == 0:
                    fch = fch_head
                else:
                    fch = fpool.tile([128, NT * C], mybir.dt.float8e4,
                                     tag="fchunk")
                    eng = nc.sync if wv_idx % 2 == 0 else nc.scalar
                    eng.dma_start(
                        fch[:],
                        params[f"feats{c}"][:, gs * c * C:(gs + NW) * c * C])
                # one-hot build: och[p, t*W+j] = (lid[p, gs*c+t] == j)
                och = fpool.tile([128, NT * W], mybir.dt.float8e4,
                                 tag="ochunk", bufs=6)
                lsl = lid_tiles[c][:, gs * c:gs * c + NT]
                in0 = bass.AP(lsl.tensor, lsl.offset,
                              [lsl.ap[0], [1, NT], [0, W]])
                in1 = bass.AP(iota_t[:].tensor, iota_t[:].offset,
                              [iota_t[:].ap[0], [0, NT], [1, W]])
                nc.vector.tensor_tensor(och[:], in0, in1,
                                        op=mybir.AluOpType.is_equal)
                # wave: even group slots in bank0, odd in bank1
                mega = psum_pool.tile([W, 1024], mybir.dt.float32, tag="ps")
                for j in range(NW // 2):
                    oa, ob = j * C, 512 + j * C
                    if c == 1:
                        ta, tb = 2 * j, 2 * j + 1
                        nc.tensor.matmul(
                            out=mega[:, oa:oa + C],
                            lhsT=och[:, ta * W:(ta + 1) * W],
                            rhs=fch[:, ta * C:(ta + 1) * C],
                            start=True, stop=True)
                        nc.tensor.matmul(
                            out=mega[:, ob:ob + C],
                            lhsT=och[:, tb * W:(tb + 1) * W],
                            rhs=fch[:, tb * C:(tb + 1) * C],
                            start=True, stop=True)
                        continue
                    for m in range(c // 2):
                        la, ra = dr_aps(och, fch, (2 * j) * c + 2 * m)
                        lb, rb = dr_aps(och, fch, (2 * j + 1) * c + 2 * m)
                        nc.tensor.matmul(
                            out=mega[:, oa:oa + C], lhsT=la, rhs=ra,
                            start=(m == 0), stop=(m == c // 2 - 1),
                            perf_mode=mybir.MatmulPerfMode.DoubleRow)
                        nc.tensor.matmul(
                            out=mega[:, ob:ob + C], lhsT=lb, rhs=rb,
                            start=(m == 0), stop=(m == c // 2 - 1),
                            perf_mode=mybir.MatmulPerfMode.DoubleRow)
                st = fpool.tile([W, NW, C], mybir.dt.bfloat16, tag="st",
                                bufs=6)
                src_ap = bass.AP(
                    mega[:].tensor, mega[:].offset,
                    [mega[:].ap[0], [512, 2], [C, NW // 2], [1, C]])
                dst_ap = bass.AP(
                    st[:].tensor, st[:].offset,
                    [st[:].ap[0], [C, 2], [2 * C, NW // 2], [1, C]])
                if wv_idx % 2 == 0:
                    nc.scalar.copy(dst_ap, src_ap)
                    nc.scalar.dma_start(
                        params[f"out{c}"][:, gs:gs + NW, :], st[:])
                else:
                    nc.vector.tensor_copy(dst_ap, src_ap)
                    nc.sync.dma_start(
                        params[f"out{c}"][:, gs:gs + NW, :], st[:])
    nc.finalize()
    _BUILD_CACHE[shape_key] = nc
    return nc


def run_scheduled(x, flat, trace=False, trace_cores=None):
    """Core pipeline given precomputed flat bins; returns (grid, results)."""
    from concourse.bass_utils import run_bass_kernel_spmd

    xflat = np.ascontiguousarray(x.reshape(-1, C))
    kept_idx = np.nonzero(flat >= 0)[0]
    fk = flat[kept_idx]
    order = np.argsort(fk, kind='stable')
    fk_sorted = fk[order]
    pidx_sorted = kept_idx[order]

    _, starts, counts = np.unique(fk_sorted, return_index=True,
                                  return_counts=True)
    qvals_sorted = _fb_quant(xflat[pidx_sorted], starts, counts)

    segs = _cut_groups(fk_sorted)
    class_split = _split_classes(segs)
    shape_key = tuple((c, class_split[c][1]) for c in CLASSES)

    maps, meta = _build_core_inputs(class_split, fk_sorted, qvals_sorted)
    nc = _build_bass(shape_key)
    res = run_bass_kernel_spmd(nc, maps, core_ids=list(range(NCORES)),
                               trace=trace, trace_cores=trace_cores)

    grid = np.zeros((NBINS + W, C), np.float32)
    for c in CLASSES:
        for ci in range(NCORES):
            outs = res.results[ci][f"out{c}"].astype(np.float32)  # [W,Gmax,C]
            bases = meta[c][ci]
            for gi in range(len(bases)):
                base = bases[gi]
                if base >= 0:
                    grid[base:base + W] += outs[:, gi]
    return grid[:NBINS], res


def kernel(x, camera2lidar_rots, camera2lidar_trans, intrins, post_rots,
           post_trans, extra_rots, extra_trans):
    x = np.asarray(x, np.float32)
    B, N = x.shape[0], x.shape[1]
    assert (B, N) == (1, 6) and x.shape[2:] == (D, FH, FW, C), x.shape

    geom = _get_geometry(
        np.asarray(camera2lidar_rots, np.float32),
        np.asarray(camera2lidar_trans, np.float32),
        np.asarray(intrins, np.float32),
        np.asarray(post_rots, np.float32),
        np.asarray(post_trans, np.float32),
        np.asarray(extra_rots, np.float32),
        np.asarray(extra_trans, np.float32),
    )
    flat = _flat_bins(geom)[0]          # [Np]
    grid, _ = run_scheduled(x, flat)
    outp = grid.reshape(NXg, NYg, C).transpose(2, 0, 1)[None]  # [1,C,NX,NY]
    return np.ascontiguousarray(outp)


# revision 13
# speedup vs baseline: 2.1399x; 1.2264x over previous
"""BEV-pool (segment-sum scatter) Trainium2 kernel for nn_BaseDepthTransform.

Design:
  Host (numpy): replicate the reference geometry -> per-point flat BEV bin id
  (depends only on the small camera matrices, not on x). Sort points by bin.
  Greedily cut the sorted stream into "groups": up to KA*128 points spanning
  < W=16 distinct bins, each group = up to KA=8 point-tiles of 128. Binary-
  decompose group tile-counts into classes {8,4,2,1} so every class has a
  uniform static schedule across the 8 SPMD cores. Ship, per core and class:
  a bf16 feature stream [128, T*C] and a u8 local-bin-id stream [128, T]
  (one byte per point; 255 for pad rows).

  Features ship as fp8e4 with per-(bin,channel) ERROR FEEDBACK applied on
  the host along the sorted stream (q_i = fp8(x_i + carry)); the device
  sums the quantized values exactly (fp8 one-hot x fp8 feats -> f32 PSUM),
  so each bin's total error collapses to the last point's residual instead
  of growing sqrt(m). Measured end-to-end rel err ~3e-3 (gate 2e-2).

  Device (Bass/Tile, SPMD x8): build the [128, W] one-hots on-chip with a
  single DVE is_equal against an iota constant (stride-0 broadcast APs),
  then per group chain c/2 DoubleRow matmuls (each contracts TWO 128-pt
  tiles, 0.5 cycles/row) accumulating the group's [16,80] segment sums in
  PSUM. Waves of up to 12 groups fill 2 PSUM banks (even slots bank0, odd
  slots bank1 so paired chains interleave); evictions PSUM->SBUF (cast to
  bf16) alternate between the Scalar and Vector engines, then DMA out.
  No dynamic addressing, no collectives.

  Host reassembly: out[group] is added into grid[base:base+16] (groups may
  share bins across classes/cores; addition commutes).
"""
import sys
sys.path.insert(0, '/opt/trn_rl_repo')

import numpy as np
import ml_dtypes

BF16 = ml_dtypes.bfloat16

# ---- static problem config (mirrors the reference) ----
IH, IW = 256, 704
FH, FW = 32, 88
D = 118
C = 80
NXg, NYg, NZg = 360, 360, 1
BXc = np.array([-53.85, -53.85, 0.0], np.float32)
DXc = np.array([0.3, 0.3, 20.0], np.float32)
NBINS = NZg * NXg * NYg  # 129600
W = 16                   # bins per group window
KA = 8                   # max tiles per group / PSUM chain
NCORES = 8
CLASSES = (8, 4, 2, 1)
WAVE = 12                # groups per PSUM wave (2 banks, 6 slots each)

_BUILD_CACHE = {}


def _frustum():
    ds = np.arange(1.0, 60.0, 0.5, dtype=np.float32)
    xs = np.linspace(0.0, IW - 1.0, FW, dtype=np.float32)
    ys = np.linspace(0.0, IH - 1.0, FH, dtype=np.float32)
    ds_g = np.broadcast_to(ds[:, None, None], (D, FH, FW))
    xs_g = np.broadcast_to(xs[None, None, :], (D, FH, FW))
    ys_g = np.broadcast_to(ys[None, :, None], (D, FH, FW))
    return np.stack([xs_g, ys_g, ds_g], axis=-1)  # [D,FH,FW,3]


def _get_geometry(c2l_rots, c2l_trans, intrins, post_rots, post_trans,
                  extra_rots, extra_trans):
    fr = _frustum()
    pts = fr[None, None] - post_trans[:, :, None, None, None, :]
    inv_pr = np.linalg.inv(post_rots).astype(np.float32)
    pts = np.einsum('bnij,bndhwj->bndhwi', inv_pr, pts).astype(np.float32)
    pts = np.concatenate([pts[..., :2] * pts[..., 2:3], pts[..., 2:3]], axis=-1)
    combine = np.einsum(
        'bnij,bnjk->bnik', c2l_rots, np.linalg.inv(intrins).astype(np.float32)
    ).astype(np.float32)
    pts = np.einsum('bnij,bndhwj->bndhwi', combine, pts).astype(np.float32)
    pts = pts + c2l_trans[:, :, None, None, None, :]
    pts = np.einsum('bij,bndhwj->bndhwi', extra_rots, pts).astype(np.float32)
    pts = pts + extra_trans[:, None, None, None, None, :]
    return pts  # [B,N,D,FH,FW,3]


def _flat_bins(geom):
    """Per-point flat bin id (int64), -1 for dropped points."""
    coords = ((geom - (BXc - DXc / 2.0)) / DXc).astype(np.int32)
    B = coords.shape[0]
    coords = coords.reshape(B, -1, 3)
    cx, cy, cz = coords[..., 0], coords[..., 1], coords[..., 2]
    kept = (cx >= 0) & (cx < NXg) & (cy >= 0) & (cy < NYg) & (cz >= 0) & (cz < NZg)
    flat = ((cz.astype(np.int64) * NXg + cx) * NYg + cy)
    flat = np.where(kept, flat, -1)
    return flat  # [B, Np]


def _round_up(x, m):
    return ((x + m - 1) // m) * m


def _cut_groups(fk_sorted):
    """Greedy: groups of <=KA*128 points spanning < W bins, binary-decomposed
    into class segments [(cls, start, npts, base), ...] in stream order."""
    n = len(fk_sorted)
    segs = []
    i = 0
    while i < n:
        hi = np.searchsorted(fk_sorted, fk_sorted[i] + W, side='left')
        j = min(i + KA * 128, hi, n)
        npts = j - i
        base = int(fk_sorted[i])
        nt = (npts + 127) // 128
        s = i
        for c in CLASSES:
            while nt >= c:
                ln = min(c * 128, j - s)
                segs.append((c, s, ln, base))
                s += ln
                nt -= c
        i = j
    return segs


def _split_classes(segs):
    """Per class: contiguous split across cores balanced by group count,
    padded to uniform per-class counts (rounded to 2 only).
    {cls: (percore seg lists, Gmax)}."""
    out = {}
    for c in CLASSES:
        cl = [s for s in segs if s[0] == c]
        G = len(cl)
        per = []
        for ci in range(NCORES):
            lo = (G * ci) // NCORES
            hi = (G * (ci + 1)) // NCORES
            per.append(cl[lo:hi])
        Gmax = max(2, _round_up(max(len(p) for p in per), 2))
        out[c] = (per, Gmax)
    return out


def _class_chunks(Gmax):
    """List of (gstart, NW) waves; NW == WAVE except an even tail."""
    chunks = []
    g = 0
    while g + WAVE <= Gmax:
        chunks.append((g, WAVE))
        g += WAVE
    if g < Gmax:
        chunks.append((g, Gmax - g))
    return chunks


def _fb_quant(vals, starts, counts):
    """fp8e4 quantization of the bin-sorted stream with per-(bin,channel)
    error feedback: q_i = fp8(x_i + carry); carry = x_i + carry - q_i.
    The shipped stream then satisfies sum_bin(q) = sum_bin(x) - last_carry."""
    FP8 = ml_dtypes.float8_e4m3
    order = np.argsort(-counts, kind='stable')  # bins by count desc
    starts_s = np.ascontiguousarray(starts[order])
    counts_s = np.ascontiguousarray(counts[order])
    maxm = int(counts_s[0]) if len(counts_s) else 0
    q = np.empty(vals.shape, FP8)
    carry = np.zeros((len(starts_s), C), np.float32)
    neg = -counts_s.astype(np.int64)
    for k in range(maxm):
        n_k = np.searchsorted(neg, -(k), side='left')  # bins with count > k
        if n_k == 0:
            break
        idx = starts_s[:n_k] + k
        v = vals[idx] + carry[:n_k]
        qk = v.astype(FP8)
        q[idx] = qk
        carry[:n_k] = v - qk.astype(np.float32)
    return q


def _build_core_inputs(class_split, fk_sorted, qvals_sorted):
    """Build per-core input dict: per class feats [128,T*C] fp8e4 and
    lids [128,T] u8 (255 = pad row)."""
    FP8 = ml_dtypes.float8_e4m3
    maps = [dict() for _ in range(NCORES)]
    meta = {c: [] for c in CLASSES}  # per class: percore array of bases
    iota = np.broadcast_to(np.arange(W, dtype=np.uint8), (128, W))
    for c in CLASSES:
        per, Gmax = class_split[c]
        T = Gmax * c
        for ci in range(NCORES):
            segs = per[ci]
            feats = np.zeros((T, 128, C), FP8)
            lids = np.full((T, 128), 255, np.uint8)
            bases = np.full((Gmax,), -1, np.int64)
            for gi, (_, s, ln, base) in enumerate(segs):
                bases[gi] = base
                lid = (fk_sorted[s:s + ln] - base).astype(np.uint8)
                t0 = gi * c
                nt = (ln + 127) // 128
                for k in range(nt):
                    a, b = k * 128, min((k + 1) * 128, ln)
                    m = b - a
                    feats[t0 + k, :m] = qvals_sorted[s + a:s + b]
                    lids[t0 + k, :m] = lid[a:b]
            maps[ci][f"feats{c}"] = np.ascontiguousarray(
                feats.transpose(1, 0, 2).reshape(128, T * C))
            maps[ci][f"lid{c}"] = np.ascontiguousarray(lids.T)
            meta[c].append(bases)
    for ci in range(NCORES):
        maps[ci]["iota16"] = np.ascontiguousarray(iota)
    return maps, meta


def _build_bass(shape_key):
    """shape_key: tuple of (cls, Gmax) pairs."""
    if shape_key in _BUILD_CACHE:
        return _BUILD_CACHE[shape_key]
    from concourse import bass, mybir, tile, bacc

    nc = bacc.Bacc()
    params = {}
    for c, Gmax in shape_key:
        T = Gmax * c
        params[f"feats{c}"] = nc.declare_dram_parameter(
            f"feats{c}", [128, T * C], mybir.dt.float8e4, isOutput=False)
        params[f"lid{c}"] = nc.declare_dram_parameter(
            f"lid{c}", [128, T], mybir.dt.uint8, isOutput=False)
        params[f"out{c}"] = nc.declare_dram_parameter(
            f"out{c}", [W, Gmax, C], mybir.dt.bfloat16, isOutput=True)
    params["iota16"] = nc.declare_dram_parameter(
        "iota16", [128, W], mybir.dt.uint8, isOutput=False)

    # interleave class waves so short small-class pipelines hide under the
    # dense class-8 stream
    chunk_order = []
    for c, Gmax in shape_key:
        chunks = _class_chunks(Gmax)
        n = len(chunks)
        for idx, (gs, nw) in enumerate(chunks):
            chunk_order.append((c, gs, nw, (idx + 0.5) / n))
    chunk_order.sort(key=lambda t: t[3])

    with tile.TileContext(nc) as tc:
        with tc.tile_pool(name="fstream", bufs=10) as fpool, \
             tc.tile_pool(name="psum", bufs=4, space="PSUM") as psum_pool:
            # issue the first feats chunk before anything else so the DMA
            # engines start pulling immediately; lids/iota follow on sync
            # while scalar covers the next chunks
            c0, gs0, NW0, _ = None, None, None, None
            fch_head = None
            if chunk_order:
                c0, gs0, NW0, _frac0 = chunk_order[0]
                fch_head = fpool.tile([128, NW0 * c0 * C], mybir.dt.float8e4,
                                      tag="fchunk", name="fch_head")
                nc.sync.dma_start(
                    fch_head[:],
                    params[f"feats{c0}"][:, gs0 * c0 * C:(gs0 + NW0) * c0 * C])
            iota_t = fpool.tile([128, W], mybir.dt.uint8, tag="iota", bufs=1)
            nc.sync.dma_start(iota_t[:], params["iota16"][:, :])
            lid_tiles = {}
            for c, Gmax in shape_key:
                lt = fpool.tile([128, Gmax * c], mybir.dt.uint8,
                                tag=f"lid{c}", bufs=1, name=f"lidt{c}")
                nc.sync.dma_start(lt[:], params[f"lid{c}"][:, :])
                lid_tiles[c] = lt

            def dr_aps(och, fch, t0):
                """lhsT/rhs APs for a DoubleRow matmul over tiles t0, t0+1."""
                lt = och[:, t0 * W:(t0 + 2) * W]
                rt = fch[:, t0 * C:(t0 + 2) * C]
                lhsT = bass.AP(lt.tensor, lt.offset,
                               [lt.ap[0], [W, 2], [1, W]])
                rhs = bass.AP(rt.tensor, rt.offset,
                              [rt.ap[0], [C, 2], [1, C]])
                return lhsT, rhs

            for wv_idx, (c, gs, NW, _frac) in enumerate(chunk_order):
                NT = NW * c
                if wv_idx == 0:
                    fch = fch_head
                else:
                    fch = fpool.tile([128, NT * C], mybir.dt.float8e4,
                                     tag="fchunk")
                    nc.sync.dma_start(
                        fch[:],
                        params[f"feats{c}"][:, gs * c * C:(gs + NW) * c * C])
                # one-hot build: och[p, t*W+j] = (lid[p, gs*c+t] == j)
                och = fpool.tile([128, NT * W], mybir.dt.float8e4,
                                 tag="ochunk", bufs=6)
                lsl = lid_tiles[c][:, gs * c:gs * c + NT]
                in0 = bass.AP(lsl.tensor, lsl.offset,
                              [lsl.ap[0], [1, NT], [0, W]])
                in1 = bass.AP(iota_t[:].tensor, iota_t[:].offset,
                              [iota_t[:].ap[0], [0, NT], [1, W]])
                nc.vector.tensor_tensor(och[:], in0, in1,
                                        op=mybir.AluOpType.is_equal)
                # wave: even group slots in bank0, odd in bank1
                mega = psum_pool.tile([W, 1024], mybir.dt.float32, tag="ps")
                for j in range(NW // 2):
                    oa, ob = j * C, 512 + j * C
                    if c == 1:
                        ta, tb = 2 * j, 2 * j + 1
                        nc.tensor.matmul(
                            out=mega[:, oa:oa + C],
                            lhsT=och[:, ta * W:(ta + 1) * W],
                            rhs=fch[:, ta * C:(ta + 1) * C],
                            start=True, stop=True)
                        nc.tensor.matmul(
                            out=mega[:, ob:ob + C],
                            lhsT=och[:, tb * W:(tb + 1) * W],
                            rhs=fch[:, tb * C:(tb + 1) * C],
                            start=True, stop=True)
                        continue
                    for m in range(c // 2):
                        la, ra = dr_aps(och, fch, (2 * j) * c + 2 * m)
                        lb, rb = dr_aps(och, fch, (2 * j + 1) * c + 2 * m)
                        nc.tensor.matmul(
                            out=mega[:, oa:oa + C], lhsT=la, rhs=ra,
                            start=(m == 0), stop=(m == c // 2 - 1),
                            perf_mode=mybir.MatmulPerfMode.DoubleRow)
                        nc.tensor.matmul(
                            out=mega[:, ob:ob + C], lhsT=lb, rhs=rb,
                            start=(m == 0), stop=(m == c // 2 - 1),
                            perf_mode=mybir.MatmulPerfMode.DoubleRow)
                st = fpool.tile([W, NW, C], mybir.dt.bfloat16, tag="st",
                                bufs=6)
                src_ap = bass.AP(
                    mega[:].tensor, mega[:].offset,
                    [mega[:].ap[0], [512, 2], [C, NW // 2], [1, C]])
                dst_ap = bass.AP(
                    st[:].tensor, st[:].offset,
                    [st[:].ap[0], [C, 2], [2 * C, NW // 2], [1, C]])
                if wv_idx % 2 == 0:
                    nc.scalar.copy(dst_ap, src_ap)
                    nc.scalar.dma_start(
                        params[f"out{c}"][:, gs:gs + NW, :], st[:])
                else:
                    nc.vector.tensor_copy(dst_ap, src_ap)
                    nc.gpsimd.dma_start(
                        params[f"out{c}"][:, gs:gs + NW, :], st[:])
    nc.finalize()
    _BUILD_CACHE[shape_key] = nc
    return nc


def run_scheduled(x, flat, trace=False, trace_cores=None):
    """Core pipeline given precomputed flat bins; returns (grid, results)."""
    from concourse.bass_utils import run_bass_kernel_spmd

    xflat = np.ascontiguousarray(x.reshape(-1, C))
    kept_idx = np.nonzero(flat >= 0)[0]
    fk = flat[kept_idx]
    order = np.argsort(fk, kind='stable')
    fk_sorted = fk[order]
    pidx_sorted = kept_idx[order]

    _, starts, counts = np.unique(fk_sorted, return_index=True,
                                  return_counts=True)
    qvals_sorted = _fb_quant(xflat[pidx_sorted], starts, counts)

    segs = _cut_groups(fk_sorted)
    class_split = _split_classes(segs)
    shape_key = tuple((c, class_split[c][1]) for c in CLASSES)

    maps, meta = _build_core_inputs(class_split, fk_sorted, qvals_sorted)
    nc = _build_bass(shape_key)
    res = run_bass_kernel_spmd(nc, maps, core_ids=list(range(NCORES)),
                               trace=trace, trace_cores=trace_cores)

    grid = np.zeros((NBINS + W, C), np.float32)
    for c in CLASSES:
        for ci in range(NCORES):
            outs = res.results[ci][f"out{c}"].astype(np.float32)  # [W,Gmax,C]
            bases = meta[c][ci]
            for gi in range(len(bases)):
                base = bases[gi]
                if base >= 0:
                    grid[base:base + W] += outs[:, gi]
    return grid[:NBINS], res


def kernel(x, camera2lidar_rots, camera2lidar_trans, intrins, post_rots,
           post_trans, extra_rots, extra_trans):
    x = np.asarray(x, np.float32)
    B, N = x.shape[0], x.shape[1]
    assert (B, N) == (1, 6) and x.shape[2:] == (D, FH, FW, C), x.shape

    geom = _get_geometry(
        np.asarray(camera2lidar_rots, np.float32),
        np.asarray(camera2lidar_trans, np.float32),
        np.asarray(intrins, np.float32),
        np.asarray(post_rots, np.float32),
        np.asarray(post_trans, np.float32),
        np.asarray(extra_rots, np.float32),
        np.asarray(extra_trans, np.float32),
    )
    flat = _flat_bins(geom)[0]          # [Np]
    grid, _ = run_scheduled(x, flat)
    outp = grid.reshape(NXg, NYg, C).transpose(2, 0, 1)[None]  # [1,C,NX,NY]
    return np.ascontiguousarray(outp)
